# revision 1
# baseline (speedup 1.0000x reference)
"""Trainium2 Bass kernel for the Basicgate multivoxel attention module.

The chain voxel-features -> attention logit is linear, so it collapses:

  logit(h,w) = sum_k T[k, h+dy_k-1, w+dx_k-1]            (point terms)
             + sum_k S[k] * gated(h+dy_k-1, w+dx_k-1)    (gated 3x3)
             + edge-constant terms                        (biases + padding)
  out = img * sigmoid(logit + sp_b)

where per point p of set i at cell (hp,wp): T[:, hp, wp] += B_i @ x_p with
B0 = V@W2@W0 (9,35), B1 = V@W2@W1 (9,67), B2 = V@W2 (9,131), x_p the
concat(feat, coord) vector, V (9,131) the 3x3 conv taps; gated = w3.img + b3
per pixel; S[k] = sum_c V[k,c].

Sharding: H split across 8 cores (32 rows each + 1 halo row per side), points
bucketed by row to the owning core(s) on the host. No collectives.

Device pipeline per core:
  A. zero a DRAM scratch T (24064 rows x 64 f32; 256B row stride required by
     dma_scatter_add)
  B. per 4096-token chunk: load x (channels on partitions, tokens on free),
     PE matmul t = x^T @ B^T (tokens on partitions, 9 taps on free),
     dma_scatter_add t into T by cell index (hardware scatter-add via SWDGE)
  C. stream img (34 rows x 2) through PE against w3 -> gated map (34,706)
  D. read T back as (34, 706*9); assemble logit in PSUM with 12 small PE
     matmuls against host-built row-shift selection matrices (E_dy / G_dx)
     -- PE does the partition (row) shifts that lane-locked DVE cannot;
     add edge-constant column fixes; sigmoid on ACT with per-row bias.
  E. per 2 rows: PE-replicate att to 128 partitions (ones(1,128) matmul),
     re-stream img, DVE multiply, DMA out.
"""

import numpy as np

# ---- problem constants (hardcoded per contract) ----
C_IMG = 256
H, W = 256, 704
CH = [32, 64, 128]
COUT = 131
N_CORES = 8
R = 32            # owned rows per core
L = 34            # local rows incl 1-row halo each side
WP = W + 2        # padded width
CELLS = L * WP    # 24004
T_ROWS = 24064    # roundup(CELLS+1, 128); rows >= CELLS are trash
TRASH = CELLS     # padding tokens scatter here (zero payload anyway)
T_STEP = 64       # f32 elements per T row (256B stride, required by HW)
# Each set gets its own 9-float slot inside the 256B row (offsets 0/9/18):
# cells are unique within a set, so no destination is ever written twice --
# immune to the HW scatter's non-accumulating RMW across writes.
CHUNK_TOK = 4096  # max tokens per scatter chunk
XHALF = 2048      # tokens per x-load tile
KS = [CH[0] + 3, CH[1] + 3, COUT]   # 35, 67, 131

LAST_RESULT = None  # stash of BassKernelResults for the test harness


def _fold_weights(inputs):
    f8 = np.float64
    W0 = inputs["rd0_w"][:, :, 0, 0].astype(f8)   # (131, 35)
    W1 = inputs["rd1_w"][:, :, 0, 0].astype(f8)   # (131, 67)
    W2 = inputs["rd2_w"][:, :, 0, 0].astype(f8)   # (131, 131)
    w3 = inputs["rd3_w"][0, :, 0, 0].astype(f8)   # (256,)
    b0 = inputs["rd0_b"].astype(f8)
    b1 = inputs["rd1_b"].astype(f8)
    b2 = inputs["rd2_b"].astype(f8)
    b3 = float(inputs["rd3_b"][0])
    spb = float(inputs["sp_b"][0])
    # V[k=dy*3+dx, c] = sp_w[0, c, dy, dx]
    V = inputs["sp_w"][0].astype(f8).transpose(1, 2, 0).reshape(9, COUT)
    B = [V @ (W2 @ W0), V @ (W2 @ W1), V @ W2]
    cc = V @ (W2 @ (b0 + b1) + b2)   # (9,)
    S = V.sum(axis=1)                # (9,)
    return dict(B=B, cc=cc, S=S, C_all=float(cc.sum()),
                w3=w3, b3=b3, spb=spb)


def _build_program(n_pad, b3):
    """Build the SPMD bass program. n_pad = per-set padded token counts
    (multiples of 128, identical across cores). b3 is the img-gate bias
    (baked immediate)."""
    import concourse.bacc as bacc
    import concourse.mybir as mybir
    import concourse.tile as tile

    f32 = mybir.dt.float32
    i16 = mybir.dt.int16
    Alu = mybir.AluOpType
    Act = mybir.ActivationFunctionType

    nc = bacc.Bacc("TRN2", target_bir_lowering=False, debug=False,
                   num_devices=N_CORES)

    img = nc.dram_tensor("img", [C_IMG, L, W], f32, kind="ExternalInput").ap()
    xs = [nc.dram_tensor(f"x{i}", [KS[i], n_pad[i]], f32,
                         kind="ExternalInput").ap() for i in range(3)]
    idxs_d = [nc.dram_tensor(f"idx{i}", [16, n_pad[i] // 16], i16,
                             kind="ExternalInput").ap() for i in range(3)]
    bts = [nc.dram_tensor(f"bt{i}", [KS[i], 9], f32,
                          kind="ExternalInput").ap() for i in range(3)]
    w3d = nc.dram_tensor("w3", [C_IMG], f32, kind="ExternalInput").ap()
    emd = nc.dram_tensor("emats", [L, 192], f32, kind="ExternalInput").ap()
    rowmaskd = nc.dram_tensor("rowmask", [L, 1], f32, kind="ExternalInput").ap()
    rowfixd = nc.dram_tensor("rowfix", [R, 1], f32, kind="ExternalInput").ap()
    colfix0d = nc.dram_tensor("colfix0", [R, 1], f32, kind="ExternalInput").ap()
    colfix1d = nc.dram_tensor("colfix1", [R, 1], f32, kind="ExternalInput").ap()
    out = nc.dram_tensor("out", [C_IMG, R, W], f32, kind="ExternalOutput").ap()

    Tds = [nc.dram_tensor(f"Tscratch{i}", [T_ROWS, T_STEP], f32)
           for i in range(3)]

    with tile.TileContext(nc) as tc:
        with (
            tc.tile_pool(name="persist", bufs=1) as pp,
            tc.tile_pool(name="work", bufs=3) as wp,
            tc.tile_pool(name="imgp", bufs=3) as ip,
            tc.tile_pool(name="psum", bufs=2, space="PSUM") as psp,
        ):
            # ---- persistent small tensors ----
            bt_t = []
            for i in range(3):
                if KS[i] <= 128:
                    t = pp.tile([KS[i], 9], f32, tag=f"bt{i}")
                    nc.sync.dma_start(out=t[:], in_=bts[i][:])
                    bt_t.append((t, None))
                else:
                    ta = pp.tile([128, 9], f32, tag=f"bt{i}a")
                    tb = pp.tile([KS[i] - 128, 9], f32, tag=f"bt{i}b")
                    nc.sync.dma_start(out=ta[:], in_=bts[i][0:128, :])
                    nc.sync.dma_start(out=tb[:], in_=bts[i][128:KS[i], :])
                    bt_t.append((ta, tb))
            w3_t = pp.tile([C_IMG // 2, 2], f32, tag="w3")
            nc.sync.dma_start(out=w3_t[:],
                              in_=w3d[:].rearrange("(hh c) -> c hh", hh=2))
            em_t = pp.tile([L, 192], f32, tag="emats")
            nc.sync.dma_start(out=em_t[:], in_=emd[:])
            ones_t = pp.tile([1, 128], f32, tag="ones")
            nc.vector.memset(ones_t[:], 1.0)
            rowmask_t = pp.tile([L, 1], f32, tag="rowmask")
            nc.sync.dma_start(out=rowmask_t[:], in_=rowmaskd[:])
            rowfix_t = pp.tile([R, 1], f32, tag="rowfix")
            nc.sync.dma_start(out=rowfix_t[:], in_=rowfixd[:])
            colfix0_t = pp.tile([R, 1], f32, tag="colfix0")
            nc.sync.dma_start(out=colfix0_t[:], in_=colfix0d[:])
            colfix1_t = pp.tile([R, 1], f32, tag="colfix1")
            nc.sync.dma_start(out=colfix1_t[:], in_=colfix1d[:])

            gmap = pp.tile([L, WP], f32, tag="gmap")
            nc.vector.memset(gmap[:], 0.0)
            T_sbs = [pp.tile([L, WP * 9], f32, tag=f"Tsb{i}", name=f"Tsb{i}")
                     for i in range(2)]
            att = pp.tile([R, W], f32, tag="att")

            # ---- stage A: zero T scratch in DRAM ----
            ZC = 1504
            ztile = pp.tile([128, ZC], f32, tag="zeros")
            nc.vector.memset(ztile[:], 0.0)
            for Td in Tds:
                t_flat = Td.ap().rearrange("r s -> (r s)").rearrange(
                    "(p n) -> p n", p=128)
                for z in range(T_ROWS * T_STEP // 128 // ZC):
                    nc.sync.dma_start(out=t_flat[:, z * ZC:(z + 1) * ZC],
                                      in_=ztile[:])

            # ---- stage B: point pipeline (per set -> private 9-col slot) ----
            for s in range(3):
                for c0 in range(0, n_pad[s], CHUNK_TOK):
                    n = min(CHUNK_TOK, n_pad[s] - c0)
                    tiles = n // 128
                    tpsum = psp.tile([128, (CHUNK_TOK // 128) * 9], f32,
                                     tag="pts")
                    ka = min(KS[s], 128)
                    for h0 in range(c0, c0 + n, XHALF):
                        cols = min(XHALF, c0 + n - h0)
                        xt = wp.tile([128, XHALF], f32, tag="x")
                        nc.sync.dma_start(out=xt[:ka, :cols],
                                          in_=xs[s][0:ka, h0:h0 + cols])
                        xtb = None
                        if KS[s] > 128:
                            xtb = wp.tile([KS[s] - 128, XHALF], f32, tag="xb")
                            nc.sync.dma_start(
                                out=xtb[:, :cols],
                                in_=xs[s][128:KS[s], h0:h0 + cols])
                        for jt in range(cols // 128):
                            po = (h0 + jt * 128 - c0) // 128 * 9
                            cx = jt * 128
                            if xtb is None:
                                nc.tensor.matmul(
                                    tpsum[:, po:po + 9],
                                    xt[:ka, cx:cx + 128],
                                    bt_t[s][0][:],
                                    start=True, stop=True)
                            else:
                                nc.tensor.matmul(
                                    tpsum[:, po:po + 9],
                                    xt[:, cx:cx + 128],
                                    bt_t[s][0][:],
                                    start=True, stop=False)
                                nc.tensor.matmul(
                                    tpsum[:, po:po + 9],
                                    xtb[:, cx:cx + 128],
                                    bt_t[s][1][:],
                                    start=False, stop=True)
                    icols = n // 16
                    idx_t = wp.tile([128, CHUNK_TOK // 16], i16, tag="idx")
                    nc.vector.memset(idx_t[:], 0)
                    nc.sync.dma_start(
                        out=idx_t[:16, :icols],
                        in_=idxs_d[s][:, c0 // 16:c0 // 16 + icols])
                    t_sb = wp.tile([128, (CHUNK_TOK // 128) * 9], f32, tag="t")
                    nc.vector.tensor_copy(out=t_sb[:, :tiles * 9],
                                          in_=tpsum[:, :tiles * 9])
                    nc.gpsimd.dma_scatter_add(
                        Tds[s].ap()[:, 0:9],
                        t_sb[:, :tiles * 9].rearrange("p (t e) -> p t e", e=9),
                        idx_t[:, :icols],
                        n,
                        n,
                        9,
                        elem_step=T_STEP,
                    )

            # ---- stage C: gated map ----
            for rc in range(L // 2):
                gpsum = psp.tile([1, 2 * W], f32, tag="big")
                imgt = []
                for hh in range(2):
                    it = ip.tile([128, 2 * W], f32, tag="img")
                    nc.sync.dma_start(
                        out=it[:],
                        in_=img[hh * 128:(hh + 1) * 128,
                                2 * rc:2 * rc + 2, :].rearrange(
                                    "c r w -> c (r w)"))
                    imgt.append(it)
                for off, n in ((0, 512), (512, 512), (1024, 384)):
                    for hh in range(2):
                        nc.tensor.matmul(
                            gpsum[:, off:off + n],
                            w3_t[:, hh:hh + 1],
                            imgt[hh][:, off:off + n],
                            start=(hh == 0), stop=(hh == 1))
                gstage = wp.tile([1, 2 * W], f32, tag="gstage")
                nc.scalar.copy(out=gstage[:], in_=gpsum[:])
                for r01 in range(2):
                    nc.scalar.dma_start(
                        out=gmap[2 * rc + r01:2 * rc + r01 + 1, 1:1 + W],
                        in_=gstage[:, r01 * W:(r01 + 1) * W])
            # gmap = (gmap + b3) * rowmask on interior columns
            nc.vector.tensor_scalar(
                out=gmap[:, 1:1 + W], in0=gmap[:, 1:1 + W],
                scalar1=float(b3), scalar2=rowmask_t[:, 0:1],
                op0=Alu.add, op1=Alu.mult)

            # ---- stage D: T readback (per set slot) + logit assembly on PE ----
            lg = psp.tile([R, W], f32, tag="big")
            segs = ((0, 512), (512, 192))
            nseg_mms = [0, 0]
            total_mms = 30  # 27 T taps + 3 gated taps per segment
            for si in range(3):
                tsb = T_sbs[si % 2]
                # split by partition group so descriptors spread across the
                # SDMA engines (a 34-partition dst concentrates on ~2)
                for g0 in range(0, L, 4):
                    gn = min(4, L - g0)
                    nc.sync.dma_start(
                        out=tsb[g0:g0 + gn, :].rearrange(
                            "h (w e) -> h w e", e=9),
                        in_=Tds[si].ap()[g0 * WP:(g0 + gn) * WP,
                                         0:9].rearrange(
                            "(h w) e -> h w e", w=WP))
                T3 = tsb[:].rearrange("h (w e) -> h w e", e=9)
                for gi, (off, n) in enumerate(segs):
                    for k in range(9):
                        dy, dx = divmod(k, 3)
                        nc.tensor.matmul(
                            lg[:, off:off + n],
                            em_t[:, dy * 32:dy * 32 + 32],
                            T3[:, dx:dx + W, k][:, off:off + n],
                            start=(nseg_mms[gi] == 0), stop=False)
                        nseg_mms[gi] += 1
            for gi, (off, n) in enumerate(segs):
                for dx in range(3):
                    nseg_mms[gi] += 1
                    nc.tensor.matmul(
                        lg[:, off:off + n],
                        em_t[:, 96 + dx * 32:96 + dx * 32 + 32],
                        gmap[:, dx:dx + W][:, off:off + n],
                        start=False, stop=(nseg_mms[gi] == total_mms))
            nc.vector.tensor_tensor(out=lg[:, 0:1], in0=lg[:, 0:1],
                                    in1=colfix0_t[:, 0:1], op=Alu.add)
            nc.vector.tensor_tensor(out=lg[:, W - 1:W], in0=lg[:, W - 1:W],
                                    in1=colfix1_t[:, 0:1], op=Alu.add)
            # rowfix carries C_all + sp_b + row-edge constants
            nc.scalar.activation(att[:], lg[:], Act.Sigmoid,
                                 bias=rowfix_t[:, 0:1], scale=1.0)

            # ---- stage E: broadcast multiply + store ----
            for rc in range(R // 2):
                a1p = wp.tile([1, 2 * W], f32, tag="a1p")
                for r01 in range(2):
                    nc.scalar.dma_start(
                        out=a1p[:, r01 * W:(r01 + 1) * W],
                        in_=att[2 * rc + r01:2 * rc + r01 + 1, :])
                attb = psp.tile([128, 2 * W], f32, tag="big")
                for off, n in ((0, 512), (512, 512), (1024, 384)):
                    nc.tensor.matmul(
                        attb[:, off:off + n],
                        ones_t[:],
                        a1p[:, off:off + n],
                        start=True, stop=True)
                for hh in range(2):
                    it = ip.tile([128, 2 * W], f32, tag="img")
                    nc.sync.dma_start(
                        out=it[:],
                        in_=img[hh * 128:(hh + 1) * 128,
                                2 * rc + 1:2 * rc + 3, :].rearrange(
                                    "c r w -> c (r w)"))
                    ot = ip.tile([128, 2 * W], f32, tag="out")
                    nc.vector.tensor_tensor(out=ot[:], in0=it[:],
                                            in1=attb[:], op=Alu.mult)
                    nc.scalar.dma_start(
                        out=out[hh * 128:(hh + 1) * 128,
                                2 * rc:2 * rc + 2, :].rearrange(
                                    "c r w -> c (r w)"),
                        in_=ot[:])

    nc.compile()
    return nc


def _prepare(inputs):
    """Host-side fold + shard. Returns (n_pad, n_tot, b3, in_maps)."""
    fold = _fold_weights(inputs)
    cc, S = fold["cc"], fold["S"]

    grids = [np.asarray(inputs[f"img_grid_{i}"]) for i in range(3)]
    feats = [np.asarray(inputs[f"voxel_feat_{i}"]) for i in range(3)]
    coords = [np.asarray(inputs[f"voxel_coord_{i}"]) for i in range(3)]
    img_feat = np.asarray(inputs["img_feat"])

    sels = []
    for c in range(N_CORES):
        lo = R * c - 1
        per = []
        for i in range(3):
            rows = grids[i][:, 1]
            per.append(np.nonzero((rows >= lo) & (rows < lo + L))[0])
        sels.append(per)

    n_pad = []
    for i in range(3):
        mx = max(len(sels[c][i]) for c in range(N_CORES))
        n_pad.append(-(-mx // 128) * 128)

    # selection matrices for the PE row-shift taps: [E0 E1 E2 G0 G1 G2]
    emats = np.zeros((L, 192), np.float32)
    for dy in range(3):
        for p in range(R):
            emats[p + dy, dy * 32 + p] = 1.0
    for dx in range(3):
        for dy in range(3):
            for p in range(R):
                emats[p + dy, 96 + dx * 32 + p] = np.float32(S[dy * 3 + dx])

    in_maps = []
    for c in range(N_CORES):
        lo = R * c - 1
        m = {}
        slab = np.zeros((C_IMG, L, W), np.float32)
        g0, g1 = max(lo, 0), min(lo + L, H)
        slab[:, g0 - lo:g1 - lo, :] = img_feat[:, g0:g1, :]
        m["img"] = slab
        for i in range(3):
            sel = sels[c][i]
            n = len(sel)
            x = np.zeros((KS[i], n_pad[i]), np.float32)
            idx_set = np.full(n_pad[i], TRASH, np.int64)
            if n:
                x[:, :n] = np.concatenate(
                    [feats[i][sel], coords[i][sel]], axis=1).T
                hl = grids[i][sel, 1].astype(np.int64) - lo
                wl = grids[i][sel, 0].astype(np.int64) + 1
                idx_set[:n] = hl * WP + wl
            m[f"x{i}"] = x
            wrapped = np.empty((16, n_pad[i] // 16), np.int64)
            ii = np.arange(n_pad[i])
            wrapped[ii % 16, ii // 16] = idx_set
            m[f"idx{i}"] = wrapped.astype(np.int16)
        for i in range(3):
            m[f"bt{i}"] = np.ascontiguousarray(
                fold["B"][i].T.astype(np.float32))
        m["w3"] = fold["w3"].astype(np.float32)
        m["emats"] = emats
        rowmask = np.zeros((L, 1), np.float32)
        rowmask[g0 - lo:g1 - lo] = 1.0
        m["rowmask"] = rowmask
        # rowfix: C_all + sp_b + row-edge constants (used as sigmoid bias)
        rowfix = np.full((R, 1), fold["C_all"] + fold["spb"], np.float64)
        colfix0 = np.full((R, 1), -(cc[0] + cc[3] + cc[6]))
        colfix1 = np.full((R, 1), -(cc[2] + cc[5] + cc[8]))
        for hloc in range(R):
            g = R * c + hloc
            if g == 0:
                rowfix[hloc] += -(cc[0] + cc[1] + cc[2])
                colfix0[hloc] += cc[0]
                colfix1[hloc] += cc[2]
            if g == H - 1:
                rowfix[hloc] += -(cc[6] + cc[7] + cc[8])
                colfix0[hloc] += cc[6]
                colfix1[hloc] += cc[8]
        m["rowfix"] = rowfix.astype(np.float32)
        m["colfix0"] = colfix0.astype(np.float32)
        m["colfix1"] = colfix1.astype(np.float32)
        in_maps.append(m)
    return n_pad, fold["b3"], in_maps


def kernel(**inputs):
    global LAST_RESULT
    from concourse.bass_utils import run_bass_kernel_spmd

    n_pad, b3, in_maps = _prepare(inputs)
    nc = _build_program(n_pad, b3)
    res = run_bass_kernel_spmd(nc, in_maps, core_ids=list(range(N_CORES)))
    LAST_RESULT = res
    out = np.concatenate(
        [res.results[c]["out"] for c in range(N_CORES)], axis=1)
    return np.ascontiguousarray(out.astype(np.float32))

